# revision 6
# baseline (speedup 1.0000x reference)
"""Trainium2 Bass kernel for nn_BinaryDiff: out = x @ base + coeff * (x @ mask).

Fused as a single matmul: out = x @ W where W = base + coeff * mask.

Sharding over 8 NeuronCores: 4 row-groups of x (2048 rows each) x 2
column-groups of W (2048 cols each). Each core computes a [2048, 2048]
tile of the [8192, 4096] output.

Per-core device kernel (two N-half passes to hide the 64 MiB W load):
  - W = base + c*mask built on-chip (one DVE scalar_tensor_tensor per
    k-tile, int32 mask consumed directly), cached in SBUF as bf16.
    Half A (cols 0:N/2) is built up front; half B's loads+builds are
    emitted interleaved into PASS A so they stream during compute.
  - PASS A: per m-tile, stage x rows fp32, transpose 128x128 blocks on
    TensorE, cast to bf16 on the PSUM->SBUF copyback (DVE), spill the
    transposed tile to DRAM, then matmul against W-half-A.
  - PASS B: stream the spilled xT tiles back (1 DMA each) and matmul
    against W-half-B. No transposes.
  - PSUM fp32 accumulation over K; output copyback on ScalarE; fp32 out.
"""

import numpy as np

import concourse.bass as bass
import concourse.mybir as mybir
import concourse.tile as tile
from concourse import bacc
from concourse.masks import make_identity

P = 128
FULL_M, FULL_K, FULL_N = 8192, 4096, 4096
ROW_SHARDS, COL_SHARDS = 4, 2
CORE_M = FULL_M // ROW_SHARDS   # 2048
CORE_N = FULL_N // COL_SHARDS   # 2048


def build_kernel(M=CORE_M, K=FULL_K, N=CORE_N, debug=False):
    """Build the per-core Bass program. All cores run the same program (SPMD)."""
    f32 = mybir.dt.float32
    i32 = mybir.dt.int32
    bf16 = mybir.dt.bfloat16

    M_T = M // P            # m-tiles of 128 rows
    K_T = K // P            # k-tiles of 128
    N_MM = 512              # matmul moving free dim (one PSUM bank)
    NH = N // 2             # N half width
    NH_T = NH // N_MM       # 512-subtiles per half
    XH = min(K, 2048)       # x staging half-width
    XH_T = K // XH

    nc = bacc.Bacc("TRN2", target_bir_lowering=False, debug=debug)

    x_d = nc.dram_tensor("x", [M, K], f32, kind="ExternalInput").ap()
    base_d = nc.dram_tensor("base", [K, N], f32, kind="ExternalInput").ap()
    mask_d = nc.dram_tensor("mask", [K, N], i32, kind="ExternalInput").ap()
    coeff_d = nc.dram_tensor("coeff", [P, 1], f32, kind="ExternalInput").ap()
    out_d = nc.dram_tensor("out", [M, N], f32, kind="ExternalOutput").ap()

    with tile.TileContext(nc) as tc:
        with (
            tc.tile_pool(name="const", bufs=1) as const,
            tc.tile_pool(name="wcache", bufs=1) as wcache,
            tc.tile_pool(name="wstage", bufs=2) as wstage,
            tc.tile_pool(name="xstage", bufs=2) as xstage,
            tc.tile_pool(name="xt", bufs=3) as xtpool,
            tc.tile_pool(name="ostage", bufs=3) as ostage,
            tc.tile_pool(name="xspill", bufs=1, space="DRAM") as xspill,
            tc.tile_pool(name="tpsum", bufs=2, space="PSUM") as tpsum,
            tc.tile_pool(name="mpsum", bufs=6, space="PSUM") as mpsum,
        ):
            ident = const.tile([P, P], f32)
            make_identity(nc, ident[:])
            c128 = const.tile([P, 1], f32)
            nc.sync.dma_start(out=c128[:], in_=coeff_d[:])

            w_a = wcache.tile([P, K_T, NH], bf16, name="w_a")
            w_b = wcache.tile([P, K_T, NH], bf16, name="w_b")
            xts = xspill.tile([M_T, P, K_T * P], bf16)

            def build_w_chunk(k, half):
                """Load base/mask k-tile for one N-half and fuse into W."""
                cs = slice(half * NH, (half + 1) * NH)
                dst = w_a if half == 0 else w_b
                bst = wstage.tile([P, NH], f32, name="bst")
                mst = wstage.tile([P, NH], i32, name="mst")
                nc.sync.dma_start(out=bst[:], in_=base_d[k * P:(k + 1) * P, cs])
                nc.sync.dma_start(out=mst[:], in_=mask_d[k * P:(k + 1) * P, cs])
                nc.vector.scalar_tensor_tensor(
                    out=dst[:, k, :],
                    in0=mst[:],
                    scalar=c128[:, 0:1],
                    in1=bst[:],
                    op0=mybir.AluOpType.mult,
                    op1=mybir.AluOpType.add,
                )

            TG = min(4, K_T)          # transposes per merged copyback group
            k_groups = [
                list(range(g, min(g + TG, K_T))) for g in range(0, K_T, TG)
            ]

            def emit_x_dma(m):
                """Stage x rows for m-tile (fp32), return list of half-tiles."""
                rs = slice(m * P, (m + 1) * P)
                halves = []
                for h in range(XH_T):
                    xst = xstage.tile([P, XH], f32, name="xst")
                    nc.gpsimd.dma_start(
                        out=xst[:], in_=x_d[rs, h * XH:(h + 1) * XH]
                    )
                    halves.append(xst)
                return halves

            def emit_t_group(halves, xt, group):
                """PE-transpose a group of k-tiles, one merged bf16 copyback."""
                pst = tpsum.tile([P, TG, P], f32)
                for j, k in enumerate(group):
                    h, kk = divmod(k, XH // P)
                    nc.tensor.transpose(
                        pst[:, j, :],
                        halves[h][:, kk * P:(kk + 1) * P],
                        ident[:],
                    )
                g0 = group[0]
                nc.vector.tensor_copy(
                    out=xt[:, g0:g0 + len(group), :], in_=pst[:, :len(group), :]
                )

            def emit_mm_group(xt, w_half, psums, group):
                for k in group:
                    for n in range(NH_T):
                        nc.tensor.matmul(
                            psums[n][:],
                            lhsT=xt[:, k, :],
                            rhs=w_half[:, k, n * N_MM:(n + 1) * N_MM],
                            start=(k == 0),
                            stop=(k == K_T - 1),
                        )

            def emit_out(psums, m, half):
                rs = slice(m * P, (m + 1) * P)
                for n in range(NH_T):
                    ob = ostage.tile([P, N_MM], f32, name="ob")
                    nc.scalar.copy(out=ob[:], in_=psums[n][:])
                    col0 = half * NH + n * N_MM
                    nc.scalar.dma_start(
                        out=out_d[rs, col0:col0 + N_MM], in_=ob[:]
                    )

            # ---- W half A up front ----
            for k in range(K_T):
                build_w_chunk(k, 0)

            # ---- PASS A (software-pipelined): m's matmul groups are
            # interleaved with (m+1)'s transpose groups so the PE never
            # sits idle behind the transpose->copyback chain. W half B
            # builds are emitted inside the loop so they stream during
            # PASS A compute. Each xT tile is spilled to DRAM for PASS B.
            wb_per_m = (K_T + M_T - 1) // M_T
            halves = emit_x_dma(0)
            xt = xtpool.tile([P, K_T, P], bf16, name="xt")
            for g in k_groups:
                emit_t_group(halves, xt, g)
            nc.gpsimd.dma_start(out=xts[0], in_=xt[:])
            for m in range(M_T):
                next_xt = None
                if m + 1 < M_T:
                    next_halves = emit_x_dma(m + 1)
                    next_xt = xtpool.tile([P, K_T, P], bf16, name="xt")
                psums = [
                    mpsum.tile([P, N_MM], f32, name="mmps") for _ in range(NH_T)
                ]
                for g in k_groups:
                    if next_xt is not None:
                        emit_t_group(next_halves, next_xt, g)
                    emit_mm_group(xt, w_a, psums, g)
                if next_xt is not None:
                    nc.gpsimd.dma_start(out=xts[m + 1], in_=next_xt[:])
                emit_out(psums, m, 0)
                for j in range(wb_per_m):
                    k = m * wb_per_m + j
                    if k < K_T:
                        build_w_chunk(k, 1)
                xt = next_xt

            # ---- PASS B: stream xT back, matmul vs W-half-B ----
            for m in range(M_T):
                xt = xtpool.tile([P, K_T, P], bf16, name="xt")
                nc.gpsimd.dma_start(out=xt[:], in_=xts[m])
                psums = [
                    mpsum.tile([P, N_MM], f32, name="mmps") for _ in range(NH_T)
                ]
                for g in k_groups:
                    emit_mm_group(xt, w_b, psums, g)
                emit_out(psums, m, 1)

    nc.compile()
    return nc


_NC_CACHE = {}


def _get_nc():
    if "nc" not in _NC_CACHE:
        _NC_CACHE["nc"] = build_kernel()
    return _NC_CACHE["nc"]


def make_in_maps(x, base, coeff, mask):
    x = np.asarray(x, dtype=np.float32)
    base = np.asarray(base, dtype=np.float32)
    mask = np.asarray(mask, dtype=np.int32)
    coeff = np.asarray(coeff, dtype=np.float32)

    B, L, D_IN = x.shape
    x2 = np.ascontiguousarray(x.reshape(B * L, D_IN))
    c128 = np.full((P, 1), coeff[0], dtype=np.float32)

    in_maps = []
    for i in range(8):
        rg, cg = i // COL_SHARDS, i % COL_SHARDS
        in_maps.append(
            {
                "x": x2[rg * CORE_M:(rg + 1) * CORE_M],
                "base": np.ascontiguousarray(
                    base[:, cg * CORE_N:(cg + 1) * CORE_N]
                ),
                "mask": np.ascontiguousarray(
                    mask[:, cg * CORE_N:(cg + 1) * CORE_N]
                ),
                "coeff": c128,
            }
        )
    return in_maps, (B, L)


def assemble(results, B, L):
    out = np.empty((B * L, FULL_N), dtype=np.float32)
    for i in range(8):
        rg, cg = i // COL_SHARDS, i % COL_SHARDS
        out[rg * CORE_M:(rg + 1) * CORE_M, cg * CORE_N:(cg + 1) * CORE_N] = (
            results[i]["out"]
        )
    return out.reshape(B, L, FULL_N)


def kernel(x, base, coeff, mask):
    from concourse.bass_utils import run_bass_kernel_spmd

    in_maps, (B, L) = make_in_maps(x, base, coeff, mask)
    nc = _get_nc()
    res = run_bass_kernel_spmd(nc, in_maps, list(range(8)))
    return assemble(res.results, B, L)


# revision 16
# speedup vs baseline: 1.0528x; 1.0528x over previous
"""Trainium2 Bass kernel for nn_BinaryDiff: out = x @ base + coeff * (x @ mask).

Fused as a single matmul: out = x @ W where W = base + coeff * mask.

Sharding over 8 NeuronCores: 4 row-groups of x (2048 rows each) x 2
column-groups of W (2048 cols each). Each core computes a [2048, 2048]
tile of the [8192, 4096] output.

Per-core device kernel (two N-half passes to hide the 64 MiB W load):
  - W = base + c*mask built on-chip (one DVE scalar_tensor_tensor per
    k-tile, int32 mask consumed directly), cached in SBUF as bf16.
    Half A (cols 0:N/2) is built up front; half B's loads+builds are
    emitted interleaved into PASS A so they stream during compute.
  - PASS A: per m-tile, stage x rows fp32, transpose 128x128 blocks on
    TensorE, cast to bf16 on the PSUM->SBUF copyback (DVE), spill the
    transposed tile to DRAM, then matmul against W-half-A.
  - PASS B: stream the spilled xT tiles back (1 DMA each) and matmul
    against W-half-B. No transposes.
  - PSUM fp32 accumulation over K; output copyback on ScalarE; fp32 out.
"""

import numpy as np

import concourse.bass as bass
import concourse.mybir as mybir
import concourse.tile as tile
from concourse import bacc
from concourse.masks import make_identity

P = 128
FULL_M, FULL_K, FULL_N = 8192, 4096, 4096
ROW_SHARDS, COL_SHARDS = 4, 2
CORE_M = FULL_M // ROW_SHARDS   # 2048
CORE_N = FULL_N // COL_SHARDS   # 2048


def build_kernel(M=CORE_M, K=FULL_K, N=CORE_N, debug=False):
    """Build the per-core Bass program. All cores run the same program (SPMD)."""
    f32 = mybir.dt.float32
    i32 = mybir.dt.int32
    bf16 = mybir.dt.bfloat16

    M_T = M // P            # m-tiles of 128 rows
    K_T = K // P            # k-tiles of 128
    N_MM = 512              # matmul moving free dim (one PSUM bank)
    NH = N // 2             # N half width
    NH_T = NH // N_MM       # 512-subtiles per half
    XH = min(K, 1024)       # x staging chunk width
    XH_T = K // XH

    nc = bacc.Bacc("TRN2", target_bir_lowering=False, debug=debug)

    x_d = nc.dram_tensor("x", [M, K], f32, kind="ExternalInput").ap()
    base_d = nc.dram_tensor("base", [K, N], f32, kind="ExternalInput").ap()
    mask_d = nc.dram_tensor("mask", [K, N], i32, kind="ExternalInput").ap()
    coeff_d = nc.dram_tensor("coeff", [P, 1], f32, kind="ExternalInput").ap()
    out_d = nc.dram_tensor("out", [M, N], f32, kind="ExternalOutput").ap()

    with tile.TileContext(nc) as tc:
        with (
            tc.tile_pool(name="const", bufs=1) as const,
            tc.tile_pool(name="wcache", bufs=1) as wcache,
            tc.tile_pool(name="wstage", bufs=2) as wstage,
            tc.tile_pool(name="xstage", bufs=2) as xstage,
            tc.tile_pool(name="xb", bufs=3) as xbpool,
            tc.tile_pool(name="xt", bufs=3) as xtpool,
            tc.tile_pool(name="ostage", bufs=3) as ostage,
            tc.tile_pool(name="xspill", bufs=1, space="DRAM") as xspill,
            tc.tile_pool(name="tpsum", bufs=3, space="PSUM") as tpsum,
            tc.tile_pool(name="mpsum", bufs=5, space="PSUM") as mpsum,
        ):
            ident = const.tile([P, P], bf16)
            make_identity(nc, ident[:])
            c128 = const.tile([P, 1], f32)
            nc.sync.dma_start(out=c128[:], in_=coeff_d[:])

            w_a = wcache.tile([P, K_T, NH], bf16, name="w_a")
            w_b = wcache.tile([P, K_T, NH], bf16, name="w_b")
            xts = xspill.tile([M_T, P, K_T * P], bf16)

            def build_w_chunk(k, half):
                """Load base/mask k-tile for one N-half and fuse into W."""
                cs = slice(half * NH, (half + 1) * NH)
                dst = w_a if half == 0 else w_b
                bst = wstage.tile([P, NH], f32, name="bst")
                mst = wstage.tile([P, NH], i32, name="mst")
                nc.sync.dma_start(out=bst[:], in_=base_d[k * P:(k + 1) * P, cs])
                nc.sync.dma_start(out=mst[:], in_=mask_d[k * P:(k + 1) * P, cs])
                nc.vector.scalar_tensor_tensor(
                    out=dst[:, k, :],
                    in0=mst[:],
                    scalar=c128[:, 0:1],
                    in1=bst[:],
                    op0=mybir.AluOpType.mult,
                    op1=mybir.AluOpType.add,
                )

            TG = min(4, K_T)          # transposes per merged copyback group
            k_groups = [
                list(range(g, min(g + TG, K_T))) for g in range(0, K_T, TG)
            ]

            def emit_x_dma(m):
                """Stage x rows fp32, cast to bf16 on ScalarE; return bf16
                chunk tiles (transposing bf16 runs 2x faster on the PE)."""
                rs = slice(m * P, (m + 1) * P)
                chunks = []
                for h in range(XH_T):
                    xst = xstage.tile([P, XH], f32, name="xst")
                    nc.gpsimd.dma_start(
                        out=xst[:], in_=x_d[rs, h * XH:(h + 1) * XH]
                    )
                    xb = xbpool.tile([P, XH], bf16, name="xb")
                    nc.scalar.copy(out=xb[:], in_=xst[:])
                    chunks.append(xb)
                return chunks

            def emit_t_group(chunks, xt, group):
                """PE-transpose a group of k-tiles, one merged copyback."""
                pst = tpsum.tile([P, TG, P], bf16)
                for j, k in enumerate(group):
                    h, kk = divmod(k, XH // P)
                    nc.tensor.transpose(
                        pst[:, j, :],
                        chunks[h][:, kk * P:(kk + 1) * P],
                        ident[:],
                    )
                g0 = group[0]
                nc.vector.tensor_copy(
                    out=xt[:, g0:g0 + len(group), :], in_=pst[:, :len(group), :]
                )

            def emit_mm_group(xt, w_half, psums, group):
                for k in group:
                    for n in range(NH_T):
                        nc.tensor.matmul(
                            psums[n][:],
                            lhsT=xt[:, k, :],
                            rhs=w_half[:, k, n * N_MM:(n + 1) * N_MM],
                            start=(k == 0),
                            stop=(k == K_T - 1),
                        )

            def emit_out(psums, m, half):
                rs = slice(m * P, (m + 1) * P)
                for n in range(NH_T):
                    ob = ostage.tile([P, N_MM], f32, name="ob")
                    nc.scalar.copy(out=ob[:], in_=psums[n][:])
                    col0 = half * NH + n * N_MM
                    nc.scalar.dma_start(
                        out=out_d[rs, col0:col0 + N_MM], in_=ob[:]
                    )

            # ---- W half A up front ----
            for k in range(K_T):
                build_w_chunk(k, 0)

            wb_per_m = (K_T + M_T - 1) // M_T
            for m in range(M_T):
                chunks = emit_x_dma(m)
                xt = xtpool.tile([P, K_T, P], bf16, name="xt")
                for g in k_groups:
                    emit_t_group(chunks, xt, g)
                nc.gpsimd.dma_start(out=xts[m], in_=xt[:])
                psums = [
                    mpsum.tile([P, N_MM], f32, name="mmps") for _ in range(NH_T)
                ]
                for g in k_groups:
                    emit_mm_group(xt, w_a, psums, g)
                emit_out(psums, m, 0)
                for j in range(wb_per_m):
                    k = m * wb_per_m + j
                    if k < K_T:
                        build_w_chunk(k, 1)

            # ---- PASS B: stream xT back, matmul vs W-half-B ----
            for m in range(M_T):
                xt = xtpool.tile([P, K_T, P], bf16, name="xt")
                nc.gpsimd.dma_start(out=xt[:], in_=xts[m])
                psums = [
                    mpsum.tile([P, N_MM], f32, name="mmps") for _ in range(NH_T)
                ]
                for g in k_groups:
                    emit_mm_group(xt, w_b, psums, g)
                emit_out(psums, m, 1)

    nc.compile()
    return nc


_NC_CACHE = {}


def _get_nc():
    if "nc" not in _NC_CACHE:
        _NC_CACHE["nc"] = build_kernel()
    return _NC_CACHE["nc"]


def make_in_maps(x, base, coeff, mask):
    x = np.asarray(x, dtype=np.float32)
    base = np.asarray(base, dtype=np.float32)
    mask = np.asarray(mask, dtype=np.int32)
    coeff = np.asarray(coeff, dtype=np.float32)

    B, L, D_IN = x.shape
    x2 = np.ascontiguousarray(x.reshape(B * L, D_IN))
    c128 = np.full((P, 1), coeff[0], dtype=np.float32)

    in_maps = []
    for i in range(8):
        rg, cg = i // COL_SHARDS, i % COL_SHARDS
        in_maps.append(
            {
                "x": x2[rg * CORE_M:(rg + 1) * CORE_M],
                "base": np.ascontiguousarray(
                    base[:, cg * CORE_N:(cg + 1) * CORE_N]
                ),
                "mask": np.ascontiguousarray(
                    mask[:, cg * CORE_N:(cg + 1) * CORE_N]
                ),
                "coeff": c128,
            }
        )
    return in_maps, (B, L)


def assemble(results, B, L):
    out = np.empty((B * L, FULL_N), dtype=np.float32)
    for i in range(8):
        rg, cg = i // COL_SHARDS, i % COL_SHARDS
        out[rg * CORE_M:(rg + 1) * CORE_M, cg * CORE_N:(cg + 1) * CORE_N] = (
            results[i]["out"]
        )
    return out.reshape(B, L, FULL_N)


def kernel(x, base, coeff, mask):
    from concourse.bass_utils import run_bass_kernel_spmd

    in_maps, (B, L) = make_in_maps(x, base, coeff, mask)
    nc = _get_nc()
    res = run_bass_kernel_spmd(nc, in_maps, list(range(8)))
    return assemble(res.results, B, L)
